# revision 1
# baseline (speedup 1.0000x reference)
"""Knowledge_Decomposition on 8 Trainium2 NeuronCores.

Pure batch-data-parallel: batch dim B=4096 is split across the 8 cores;
the small per-encoder weights are replicated. Per shard, one jitted
program computes both encoders:
  g = LN(pfeat @ Wg[e].T), p = LN(gfeat @ Wp[e].T)
  out[e] = p*path_att + g*geno_att  (sigmoid attention, fused dots)
"""
import numpy as np
import jax
import jax.numpy as jnp

B, L, D = 4096, 16, 256
NCORES = 8
BPC = B // NCORES

_cache = {}


def _estimator_both(gin, pin, Wg, bg, gng, gnb, Wp, bp, png, pnb, wga, bga,
                    wpa, bpa):
    # gin/pin: [bpc, L, D]; params stacked [2, ...]
    def ln(x, gamma, beta, eps=1e-5):
        m = jnp.mean(x, axis=-1, keepdims=True)
        v = jnp.mean(jnp.square(x - m), axis=-1, keepdims=True)
        return (x - m) * jax.lax.rsqrt(v + eps) * gamma + beta

    outs = []
    for e in range(2):
        g = ln(jnp.einsum('bld,ed->ble', gin, Wg[e]) + bg[e], gng[e], gnb[e])
        p = ln(jnp.einsum('bld,ed->ble', pin, Wp[e]) + bp[e], png[e], pnb[e])
        geno = jax.nn.sigmoid(
            g * jnp.einsum('bld,d->bl', p, wga[e])[..., None] + bga[e])
        path = jax.nn.sigmoid(
            p * jnp.einsum('bld,d->bl', g, wpa[e])[..., None] + bpa[e])
        outs.append(p * path + g * geno)
    return jnp.stack(outs)  # [2, bpc, L, D]


def kernel(**inputs):
    devs = jax.devices()[:NCORES]
    if "fn" not in _cache:
        _cache["fn"] = [jax.jit(_estimator_both, device=d) for d in devs]

    gfeat = np.asarray(inputs["gfeat"], np.float32)
    pfeat = np.asarray(inputs["pfeat"], np.float32)
    params = [np.asarray(inputs[k], np.float32) for k in
              ("Wg", "bg", "gng", "gnb", "Wp", "bp", "png", "pnb",
               "wga", "bga", "wpa", "bpa")]

    futs = []
    for c, d in enumerate(devs):
        bs = slice(c * BPC, (c + 1) * BPC)
        # reference calls estimator with swapped inputs: (pfeat, gfeat)
        futs.append(_cache["fn"][c](pfeat[bs], gfeat[bs], *params))
    parts = [np.asarray(f) for f in futs]
    full = np.concatenate(parts, axis=1)  # [2, B, L, D]
    return full[0], full[1]

